# revision 23
# baseline (speedup 1.0000x reference)
"""Trainium2 Bass kernel for CoordsSelect (batched voxel-feature gather).

reference semantics:
  volume: [B=4, F=16, D=120, D, D] f32, coords: [B, 3*A=6144] f32,
  num_atoms: [B] int32
  vox = floor(coords_xyz) (clipped to [0,119]); flat = ix*D*D + iy*D + iz
  out[b, f, a] = volume[b, f].flat[flat[b, a]] * (a < num_atoms[b])

Sharding: 8 cores = 4 batches x 2 feature-halves. Core c handles
batch c//2, features 8*(c%2) .. 8*(c%2)+8, all 2048 atoms.

Per-core algorithm (all on device):
  1. compute flat voxel ids from coords (exact floor via int-cast roundtrip)
  2. per feature, dma_gather the aligned 64-element (256B) window holding
     each atom's voxel: row id w = flat >> 6 (27000 rows per feature, fits
     int16); 2048 windows per call
  3. select element (flat & 63) from each window with a one-hot multiply +
     reduce on DVE; invalid atoms (a >= num_atoms) get their one-hot pushed
     out of range so they produce exact 0
  4. write [8, 2048] f32 back, 64B-contiguous per (feature, atom block)

dma_gather index wrap (per HW/ucode semantics): index position i lives at
idxs[i % 16, i // 16] (replicated across the 8 16-partition groups), and
gather output row i lands at out[i % 128, i // 128, :]. We assign position
i the atom a(i) = (i%16)*128 + ((i%128)//16)*16 + (i//128), which makes:
  - idxs[p, c] = w_tile[p, (c%8)*16 + c//8]   (pure free-dim permutation of
    the natural chunk-per-partition tile w_tile[p, m] = w(atom (p%16)*128+m))
  - gather out[p, j] = atom base(p) + j with base(p) = (p%16)*128+(p//16)*16
    i.e. 16 consecutive atoms per partition -> the within-window selector
    comes from one contiguous coords re-load (crd2), and the final DRAM
    write is 64B-contiguous runs.
"""

import numpy as np

import concourse.bass as bass
import concourse.mybir as mybir
import concourse.tile as tile
from concourse import bacc, library_config
from concourse.bass_utils import run_bass_kernel_spmd

B, F, D = 4, 16, 120
A = 2048
D3 = D * D * D          # 1_728_000
FC = F // 2             # 8 features per core
NROWS = D3 // 64        # 27_000 aligned 64-elem rows per feature
N_CORES = 8

f32 = mybir.dt.float32
i32 = mybir.dt.int32
i16 = mybir.dt.int16
Alu = mybir.AluOpType
AxisX = mybir.AxisListType.X


def _floor_nonneg(nc, pool, out, comp, ti, cc, name):
    """out = floor(comp) for comp >= 0, robust to the cast rounding mode:
    i = int(comp); c2 = float(i); out = c2 - (c2 > comp)."""
    tmp = pool.tile(list(out.shape), f32, name=f"{name}_gt")
    nc.vector.tensor_copy(out=ti[:], in_=comp)
    nc.vector.tensor_copy(out=cc[:], in_=ti[:])
    nc.vector.tensor_tensor(out=tmp[:], in0=cc[:], in1=comp, op=Alu.is_gt)
    nc.vector.tensor_tensor(out=out[:], in0=cc[:], in1=tmp[:], op=Alu.subtract)


def _flat_from_coords(nc, pool, crd_view, n, name):
    """crd_view: [128, n, 3] coords view -> returns [128, n] f32 flat ids.

    Strided (stride-3) DVE reads run ~6x slower than contiguous, so first
    compact each coordinate into a contiguous tile, then run the floor
    chain at full rate."""
    fl = pool.tile([128, n], f32, name=f"{name}_fl")
    ti = pool.tile([128, n], i32, name=f"{name}_ti")
    cc = pool.tile([128, n], f32, name=f"{name}_cc")
    acc = pool.tile([128, n], f32, name=f"{name}_acc")
    comp = pool.tile([128, n], f32, name=f"{name}_comp")
    for d_i in range(3):
        nc.vector.tensor_copy(out=comp[:], in_=crd_view[:, :, d_i : d_i + 1])
        _floor_nonneg(
            nc, pool, cc if d_i else acc, comp[:], ti, fl, f"{name}{d_i}"
        )
        if d_i == 0:
            # acc holds floor(x); scale by D
            nc.vector.tensor_scalar(
                acc[:], acc[:], float(D), None, op0=Alu.mult
            )
        else:
            nc.vector.tensor_tensor(out=acc[:], in0=acc[:], in1=cc[:], op=Alu.add)
            if d_i == 1:
                nc.vector.tensor_scalar(
                    acc[:], acc[:], float(D), None, op0=Alu.mult
                )
    nc.vector.tensor_copy(out=fl[:], in_=acc[:])
    return fl


def build_bass(debug_dumps=False):
    """Build + compile the per-core Bass program (identical on all cores)."""
    nc = bacc.Bacc(
        "TRN2",
        target_bir_lowering=False,
        debug=False,
        num_devices=N_CORES,
    )

    vol = nc.dram_tensor("vol", [FC * D3], f32, kind="ExternalInput")
    crd = nc.dram_tensor("crd", [3 * A], f32, kind="ExternalInput")
    nat = nc.dram_tensor("nat", [128], i32, kind="ExternalInput")
    # host-provided constants (like identity matrices): atom ids in the
    # gather-output layout, and the repeating 0..63 ramp for the one-hot
    am0 = nc.dram_tensor("am0", [128, 16], f32, kind="ExternalInput")
    ce = nc.dram_tensor("ce", [128, 1024], f32, kind="ExternalInput")
    out = nc.dram_tensor("out", [FC, A], f32, kind="ExternalOutput")

    with tile.TileContext(nc) as tc:
        with (
            tc.tile_pool(name="p", bufs=1) as pool,
            tc.tile_pool(name="gp", bufs=3) as gpool,
            tc.tile_pool(name="sp", bufs=2) as spool,
        ):
            # dma_gather / dma_scatter_add live in the 'mlp' Q7 ucode
            # library; load it first (the Pool engine has no earlier work).
            nc.gpsimd.load_library(library_config.mlp)
            # ---- coords, natural chunk layout: partition p holds the 128
            # atoms of chunk p%16 (replicated across the 8 groups via a
            # step-0 outer dim in the DRAM-side AP) ----
            crd_t = pool.tile([128, 3 * 128], f32)
            nc.sync.dma_start(
                crd_t[:], bass.AP(crd, 0, [[0, 8], [384, 16], [1, 384]])
            )

            cv = crd_t[:].rearrange("p (a d) -> p a d", d=3)
            fl = _flat_from_coords(nc, pool, cv, 128, "a")

            # w_tile[p, m] = fl >> 6  (aligned 256B row id, < 27000)
            vsc = pool.tile([128, 128], f32)
            nc.vector.tensor_scalar(
                vsc[:], fl[:], 1.0 / 64.0, None, op0=Alu.mult
            )
            w_t = pool.tile([128, 128], f32)
            w_ti = pool.tile([128, 128], i32)
            w_cc = pool.tile([128, 128], f32)
            _floor_nonneg(nc, pool, w_t, vsc[:], w_ti, w_cc, "w")

            # idxs[p, c] = w_tile[p, (c%8)*16 + c//8], cast to int16
            idxs = pool.tile([128, 128], i16)
            nc.vector.tensor_copy(
                out=idxs[:].rearrange("p (ch c8) -> p ch c8", c8=8),
                in_=w_t[:].rearrange("p (c8 ch) -> p ch c8", c8=8),
            )

            # ---- coords, gather-output layout: partition p holds the 16
            # consecutive atoms starting at base(p) = (p%16)*128+(p//16)*16 ----
            crd2_t = pool.tile([128, 48], f32)
            nc.scalar.dma_start(
                crd2_t[:],
                bass.AP(crd, 0, [[48, 8], [384, 16], [1, 48]]),
            )
            cv2 = crd2_t[:].rearrange("p (a d) -> p a d", d=3)
            fl2 = _flat_from_coords(nc, pool, cv2, 16, "b")

            v2 = pool.tile([128, 16], f32)
            nc.vector.tensor_scalar(v2[:], fl2[:], 1.0 / 64.0, None, op0=Alu.mult)
            w2 = pool.tile([128, 16], f32)
            w2_ti = pool.tile([128, 16], i32)
            w2_cc = pool.tile([128, 16], f32)
            _floor_nonneg(nc, pool, w2, v2[:], w2_ti, w2_cc, "w2")
            within = pool.tile([128, 16], f32)
            nc.vector.tensor_scalar(w2[:], w2[:], -64.0, None, op0=Alu.mult)
            nc.vector.tensor_tensor(
                out=within[:], in0=fl2[:], in1=w2[:], op=Alu.add
            )

            # ---- invalid-atom mask folded into the selector: atom id
            # a(p,j) = base(p) + j (the am0 const); if a >= num_atoms push
            # the selector out of the one-hot's [0,64) range ----
            am0_t = pool.tile([128, 16], f32)
            nc.scalar.dma_start(am0_t[:], am0.ap())
            nat_t = pool.tile([128, 1], i32)
            nc.scalar.dma_start(nat_t[:], nat.ap()[:, None])
            natf = pool.tile([128, 1], f32)
            nc.vector.tensor_copy(out=natf[:], in_=nat_t[:])
            pen = pool.tile([128, 16], f32)
            nc.vector.tensor_tensor(
                out=pen[:], in0=am0_t[:],
                in1=natf[:].to_broadcast([128, 16]), op=Alu.is_ge,
            )
            nc.vector.tensor_scalar(pen[:], pen[:], 65.0, None, op0=Alu.mult)
            nc.vector.tensor_tensor(
                out=within[:], in0=within[:], in1=pen[:], op=Alu.add
            )

            # one-hot selector oh[p, j, e] = (e == within[p, j])
            iota_e = pool.tile([128, 16, 64], f32)
            nc.scalar.dma_start(
                iota_e[:], ce.ap().rearrange("p (j e) -> p j e", e=64)
            )
            oh = pool.tile([128, 16, 64], f32)
            nc.vector.tensor_tensor(
                out=oh[:], in0=iota_e[:],
                in1=within[:].rearrange("p (j e) -> p j e", e=1).to_broadcast(
                    [128, 16, 64]
                ),
                op=Alu.is_equal,
            )

            # ---- per-feature gather + select + write ----
            # per-feature result tiles and per-(feature, hi) writes: every
            # feature's select and output DMA overlaps later gathers, so only
            # the last feature's ~3us select chain sits in the kernel tail.
            for f_i in range(FC):
                g_out = gpool.tile([128, 16, 64], f32, name="g_out")
                nc.gpsimd.dma_gather(
                    out_ap=g_out[:],
                    in_ap=bass.AP(vol, f_i * D3, [[64, NROWS], [1, 64]]),
                    idxs_ap=idxs[:],
                    num_idxs=A,
                    num_idxs_reg=A,
                    elem_size=64,
                    # >64 descriptors per Q7 core overflows the 16KB SBUF
                    # descriptor carveout in single-packet mode; use the
                    # ring-reclaim path instead.
                    single_packet=False,
                )
                sel = spool.tile([128, 16, 64], f32, name="sel")
                nc.vector.tensor_tensor(
                    out=sel[:], in0=g_out[:], in1=oh[:], op=Alu.mult
                )
                res_f = spool.tile([128, 16], f32, name="res_f")
                nc.vector.tensor_reduce(
                    out=res_f[:], in_=sel[:], axis=AxisX, op=Alu.add
                )
                # out[f, base(p)+j] = res_f[p, j]
                for hi_i in range(8):
                    eng = nc.sync if hi_i % 2 == 0 else nc.scalar
                    eng.dma_start(
                        bass.AP(
                            out,
                            f_i * A + hi_i * 16,
                            [[128, 16], [1, 16]],
                        ),
                        res_f[16 * hi_i : 16 * (hi_i + 1), :],
                    )

            if debug_dumps:
                d_idxs = nc.dram_tensor(
                    "d_idxs", [128, 128], i16, kind="ExternalOutput"
                )
                nc.sync.dma_start(d_idxs.ap(), idxs[:])
                d_within = nc.dram_tensor(
                    "d_within", [128, 16], f32, kind="ExternalOutput"
                )
                nc.sync.dma_start(d_within.ap(), within[:])
                d_fl = nc.dram_tensor(
                    "d_fl", [128, 128], f32, kind="ExternalOutput"
                )
                nc.sync.dma_start(d_fl.ap(), fl[:])
                d_w = nc.dram_tensor(
                    "d_w", [128, 128], f32, kind="ExternalOutput"
                )
                nc.sync.dma_start(d_w.ap(), w_t[:])

    nc.compile()
    return nc


_NC_CACHE = None


def _get_nc():
    global _NC_CACHE
    if _NC_CACHE is None:
        _NC_CACHE = build_bass()
    return _NC_CACHE


def _consts():
    p = np.arange(128)
    base = (p % 16) * 128 + (p // 16) * 16
    am0 = (base[:, None] + np.arange(16)[None, :]).astype(np.float32)
    ce = np.tile(
        np.tile(np.arange(64, dtype=np.float32), 16)[None, :], (128, 1)
    )
    return am0, ce


def make_in_maps(volume, coords, num_atoms):
    am0, ce = _consts()
    in_maps = []
    for c in range(N_CORES):
        b, fh = c // 2, c % 2
        in_maps.append(
            {
                "vol": np.ascontiguousarray(
                    volume[b, fh * FC : (fh + 1) * FC]
                ).reshape(-1),
                "crd": np.ascontiguousarray(coords[b]),
                "nat": np.full((128,), num_atoms[b], dtype=np.int32),
                "am0": am0,
                "ce": ce,
            }
        )
    return in_maps


def kernel(volume, coords, num_atoms):
    volume = np.asarray(volume, dtype=np.float32)
    coords = np.asarray(coords, dtype=np.float32)
    num_atoms = np.asarray(num_atoms, dtype=np.int32)

    nc = _get_nc()
    in_maps = make_in_maps(volume, coords, num_atoms)
    r = run_bass_kernel_spmd(nc, in_maps, core_ids=list(range(N_CORES)))

    out = np.empty((B, F, A), dtype=np.float32)
    for c, res in enumerate(r.results):
        b, fh = c // 2, c % 2
        out[b, fh * FC : (fh + 1) * FC] = res["out"]
    return out
